# revision 23
# baseline (speedup 1.0000x reference)
"""KD feature-level smooth-L1 loss kernel for Trainium2 (8 NeuronCores).

Math (per batch sample b over (C,H,W) = 256*64*64 = N elements):
  t_norm = (t - mean) * rsqrt(var + eps)          # LayerNorm, no affine
  d   = |t_norm - s|
  kd  = where(d <= 2, d*d/4, d - 1)               # smooth-L1, beta=2
  out = mean_b( sum_chw(kd) )

Schedule: the kernel is a single ordered HBM stream on the SWDGE queue
(~33.6 MB/core read at ~420 GB/s, fp32->bf16 cast in the DMA datapath).
All four teacher tiles stream FIRST (sample 3 leading, so its LayerNorm
scalars are ready ~23 us in); per-sample stats run as chunks arrive
(sum(t) via bf16 ones-matmuls on PE, sum(t^2) via ACT Square with free
accumulation).  The normalize pass tn = t*rs + (-mean*rs) runs on the
DVE (tensor_scalar, 2x bf16 rate) for samples 3,0,1 during the teacher
phase while the DVE is otherwise idle, and on ACT (Identity with
per-partition scale/bias APs) for the last sample 2, filling ACT's
idle window after its squares.  Student chunks stream second (order
s3,s0,s1,s2) and the student phase is pure KD work on the DVE:
  y = tn - s ; c = clamp(y, -2, 2) ; 4*kd = c*(2y - c)   (+ accum)
tracking chunk arrivals with ~2 us of slack; the last sample's final
chunks are halved so only ~1 us of KD trails the last input byte.
Sharding: pure data parallel, 4 samples per core; host combines.
"""

from contextlib import ExitStack
from operator import add as _operator_add

import numpy as np

import concourse.mybir as mybir
import concourse.tile as tile
from concourse import bacc
from concourse.bass_utils import run_bass_kernel_spmd

B, C, H, W = 32, 256, 64, 64
N_CORES = 8
BPC = B // N_CORES            # samples per core
P = 128
N = C * H * W                 # 1048576 elements per sample
FD = N // P                   # 8192 free-dim per partition
HF = FD // 2                  # teacher DMA half width
SC = 2048                     # student chunk width (4 per sample)
MM = 512                      # matmul free-dim block (PSUM bank width)
EPS = 1e-5
BETA = 2.0
LOSS_WEIGHT = 1.0

f32 = mybir.dt.float32
bf16 = mybir.dt.bfloat16
AF = mybir.ActivationFunctionType
OP = mybir.AluOpType
AX = mybir.AxisListType

T_ORDER = [3, 0, 1, 2]        # teacher stream order (sample 3 first)
# student stream leads with the FIRST teacher sample, whose LayerNorm
# scalars have been ready for ~20 us by the time its chunks arrive.
S_ORDER = [3, 0, 1, 2]
# per-sample student chunk widths; last sample ends with two half chunks
S_CHUNKS = {3: [SC] * 4, 0: [SC] * 4, 1: [SC] * 4, 2: [SC, SC, SC, SC // 2, SC // 2]}


# ---------------- custom fused DVE op: accum += 4*kd ----------------------
def _register_kd_op():
    import concourse.dve_ops as dve_ops
    from concourse.dve_ops import DveOp
    from concourse.dve_spec import (
        C2,
        Latch,
        Spec,
        Src0,
        Src1,
        Zero,
        _has_src1,
        lower,
        maxx,
        minn,
    )
    from concourse.dve_table_gen import dve_ver_for
    from concourse.dve_uop import DveOpSpec

    name = "TENSOR_KD_SMOOTHL1"
    if name in dve_ops._SUB_OPCODE_FOR_NAME:
        return next(op for op in dve_ops.OPS if op.name == name)

    # in0 = tn (= t*rs - mean*rs, prescaled), in1 = s.
    # y = tn - s ; c = clamp(y, -2, 2) ; body = c*(2y - c) = 4*kd
    # 6 ALU ops + 1 accum stage <= 8-stage DVE pipeline.
    y = Src0 - Src1
    c = maxx(minn(y, C2), Latch(Zero - C2))
    body = c * (y + y - c)

    def _ref(in0, in1, c0, c1, c2):
        yv = in0.astype(np.float32) - in1.astype(np.float32)
        cv = np.clip(yv, -c2, c2)
        b = (cv * (2.0 * yv - cv)).astype(np.float32)
        return b, b.reshape(b.shape[0], -1).sum(axis=-1, keepdims=True)

    spec = Spec(body=body, accum=_operator_add, reference=_ref)
    ver = dve_ver_for("TRN2")
    row = max(dve_ops._SUB_OPCODE_FOR_NAME.values()) + 1
    assert row < 0x20
    probe = DveOpSpec(
        name=name, opcode=row, uops=lower(spec, ver=ver), rd1_en=_has_src1(spec)
    )
    op = DveOp(name, spec, subdim=False, uops_sha={ver: probe.sha(ver)})
    dve_ops.OPS.append(op)
    dve_ops.CUSTOM_DVE_SPECS[name] = spec
    dve_ops._SUB_OPCODE_FOR_NAME[name] = row
    return op


KD_OP = _register_kd_op()


def _build_kernel(ctx: ExitStack, tc: "tile.TileContext", out_ap, teacher, stu):
    nc = tc.nc

    const_pool = ctx.enter_context(tc.tile_pool(name="const", bufs=1))
    t_pool = ctx.enter_context(tc.tile_pool(name="t", bufs=BPC))
    tn_pool = ctx.enter_context(tc.tile_pool(name="tn", bufs=BPC))
    s_pool = ctx.enter_context(tc.tile_pool(name="s", bufs=8))
    dead_pool = ctx.enter_context(tc.tile_pool(name="dead", bufs=1))
    kdout_pool = ctx.enter_context(tc.tile_pool(name="kdout", bufs=1))
    sums_pool = ctx.enter_context(tc.tile_pool(name="sums", bufs=BPC))
    tiny_pool = ctx.enter_context(tc.tile_pool(name="tiny", bufs=BPC))
    bc_pool = ctx.enter_context(tc.tile_pool(name="bc", bufs=BPC))
    ps_t_pool = ctx.enter_context(tc.tile_pool(name="ps_t", bufs=1, space="PSUM"))
    ps_sq_pool = ctx.enter_context(tc.tile_pool(name="ps_sq", bufs=2, space="PSUM"))
    ps_kd_pool = ctx.enter_context(tc.tile_pool(name="ps_kd", bufs=2, space="PSUM"))
    ps_bc_pool = ctx.enter_context(tc.tile_pool(name="ps_bc", bufs=2, space="PSUM"))

    ones_bf = const_pool.tile([P, 1], bf16)
    nc.vector.memset(ones_bf[:], 1.0)
    ones_f32 = const_pool.tile([P, 1], f32)
    nc.vector.memset(ones_f32[:], 1.0)
    ones_row = const_pool.tile([1, P], f32)
    nc.vector.memset(ones_row[:], 1.0)
    staging = const_pool.tile([1, 8 * BPC], f32)
    nc.vector.memset(staging[:], 0.0)
    warm = const_pool.tile([1, 2], f32)
    # touch Sqrt first so its ACT table set (which also holds Square and
    # Identity) is loaded once at kernel start, off the critical path
    nc.scalar.activation(warm[0:1, 0:1], ones_f32[0:1, 0:1], AF.Sqrt)
    nc.scalar.activation(warm[0:1, 1:2], ones_f32[0:1, 0:1], AF.Identity)

    # ---------------- input DMAs up front, ONE ordered SWDGE stream ------
    # teacher first (sample stats must complete before any loss work),
    # sample 3 leading so its scalars are ready long before the student
    # phase; then student chunks in the same sample order.
    t_tiles = {}
    for b in T_ORDER:
        t_sb = t_pool.tile([P, FD], bf16, name="t")
        # last teacher tile streams in quarters so its stats lag the
        # stream by one quarter, not one half
        nq = 4 if b == T_ORDER[-1] else 2
        qw = FD // nq
        for q in range(nq):
            nc.gpsimd.dma_start(
                t_sb[:, q * qw : (q + 1) * qw], teacher[b, :, q * qw : (q + 1) * qw]
            )
        t_tiles[b] = t_sb
    s_bufs = {}
    for b in S_ORDER:
        off = 0
        for c, w in enumerate(S_CHUNKS[b]):
            s_sb = s_pool.tile([P, SC], bf16, name="s")
            nc.gpsimd.dma_start(s_sb[:, 0:w], stu[b, :, off : off + w])
            s_bufs[(b, c)] = s_sb
            off += w

    state = {}

    def stats(b):
        t_sb = t_tiles[b]
        ps_t = ps_t_pool.tile([1, MM], f32)
        nmm = FD // MM
        for k in range(nmm):
            nc.tensor.matmul(
                ps_t[:, :],
                ones_bf[:, :],
                t_sb[:, k * MM : (k + 1) * MM],
                start=(k == 0),
                stop=(k == nmm - 1),
            )
        # sums cols 0:4 = per-chunk sum(t^2), 4:4+nch = sum(4kd) chunks
        sums = sums_pool.tile([P, 12], f32)
        for ci in range(4):
            sl = slice(ci * SC, (ci + 1) * SC)
            dead = dead_pool.tile([P, SC], bf16)
            nc.scalar.activation(
                dead[:], t_sb[:, sl], AF.Square, accum_out=sums[:, ci : ci + 1]
            )
        state[b] = {"ps_t": ps_t, "sums": sums}

    def tiny(b):
        st_ = state[b]
        bb = tiny_pool.tile([1, 24], f32)
        ps_sq = ps_sq_pool.tile([1, 4], f32)
        nc.tensor.matmul(
            ps_sq[:, :], ones_f32[:, :], st_["sums"][:, 0:4], start=True, stop=True
        )
        st = bb[0:1, 3:4]
        nc.vector.reduce_sum(out=st, in_=st_["ps_t"][:, :], axis=AX.X)
        stt = bb[0:1, 4:5]
        nc.vector.reduce_sum(out=stt, in_=ps_sq[0:1, 0:4], axis=AX.X)
        mean = bb[0:1, 2:3]
        nc.vector.tensor_scalar(mean, st, 1.0 / N, None, op0=OP.mult)
        e2 = bb[0:1, 5:6]
        nc.vector.tensor_scalar(e2, stt, 1.0 / N, EPS, op0=OP.mult, op1=OP.add)
        msq = bb[0:1, 6:7]
        nc.vector.tensor_tensor(msq, mean, mean, op=OP.mult)
        ve = bb[0:1, 7:8]
        nc.vector.tensor_tensor(ve, e2, msq, op=OP.subtract)
        inv_ve = bb[0:1, 8:9]
        nc.vector.reciprocal(inv_ve, ve)
        rs = bb[0:1, 9:10]
        nc.scalar.activation(rs, inv_ve, AF.Sqrt)  # rs0 ~= 1/sqrt(ve) (table)
        # one Newton iteration: rs <- rs*(1.5 - 0.5*ve*rs^2)
        r2 = bb[0:1, 10:11]
        nc.vector.tensor_tensor(r2, rs, rs, op=OP.mult)
        pv = bb[0:1, 11:12]
        nc.vector.tensor_tensor(pv, r2, ve, op=OP.mult)
        hh = bb[0:1, 12:13]
        nc.vector.tensor_scalar(hh, pv, -0.5, 1.5, op0=OP.mult, op1=OP.add)
        rs_f = bb[0:1, 0:1]
        nc.vector.tensor_tensor(rs_f, rs, hh, op=OP.mult)
        # bb col0 = rs (final); col1 = -mean*rs
        mean_rs = bb[0:1, 13:14]
        nc.vector.tensor_tensor(mean_rs, mean, rs_f, op=OP.mult)
        nc.vector.tensor_scalar(bb[0:1, 1:2], mean_rs, -1.0, None, op0=OP.mult)
        # broadcast rs / -mean*rs to all partitions via a PE ones-row
        # matmul, then land them in SBUF (ACT scale/bias must be SBUF APs)
        ps_bc = ps_bc_pool.tile([P, 2], f32)
        nc.tensor.matmul(
            ps_bc[:, :], ones_row[:, :], bb[0:1, 0:2], start=True, stop=True
        )
        bc_sb = bc_pool.tile([P, 2], f32)
        nc.vector.tensor_copy(bc_sb[:], ps_bc[:])
        # guard: keep ps_t (bufs=1) alive until the broadcast lands, so the
        # next sample's 16 stats matmuls cannot be scheduled ahead of this
        # sample's small PE ops (sm/bc) on the in-order PE queue.
        nc.vector.tensor_tensor(
            bb[0:1, 19:20], st_["ps_t"][0:1, 0:1], bc_sb[0:1, 0:1], op=OP.add
        )
        st_["rs_vec"] = bc_sb[:, 0:1]
        st_["nmrs_vec"] = bc_sb[:, 1:2]

    def ts_pass(b, engine):
        # tn = t*rs + (-mean*rs), per student-chunk slice
        st_ = state[b]
        t_sb = t_tiles[b]
        tn = tn_pool.tile([P, FD], bf16, name="tn")
        for c in range(4):
            sl = slice(c * SC, (c + 1) * SC)
            if engine == "act":
                nc.scalar.activation(
                    tn[:, sl],
                    t_sb[:, sl],
                    AF.Identity,
                    bias=st_["nmrs_vec"],
                    scale=st_["rs_vec"],
                )
            else:
                nc.vector.tensor_scalar(
                    tn[:, sl],
                    t_sb[:, sl],
                    st_["rs_vec"],
                    st_["nmrs_vec"],
                    op0=OP.mult,
                    op1=OP.add,
                )
        st_["tn"] = tn

    def loss(b):
        st_ = state[b]
        sums = st_["sums"]
        off = 0
        for c, w in enumerate(S_CHUNKS[b]):
            kdo = kdout_pool.tile([P, SC], bf16, name="kdo")
            nc.vector._custom_dve(
                KD_OP,
                out=kdo[:, 0:w],
                in0=st_["tn"][:, off : off + w],
                in1=s_bufs[(b, c)][:, 0:w],
                imm2=BETA,
                accum_out=sums[:, 4 + c : 5 + c],
            )
            off += w

    def final(b):
        st_ = state[b]
        nch = len(S_CHUNKS[b])
        ps_kd = ps_kd_pool.tile([1, 8], f32)
        nc.tensor.matmul(
            ps_kd[:, 0:nch],
            ones_f32[:, :],
            st_["sums"][:, 4 : 4 + nch],
            start=True,
            stop=True,
        )
        nc.vector.reduce_sum(
            out=staging[0:1, 8 * b : 8 * b + 1], in_=ps_kd[0:1, 0:nch], axis=AX.X
        )

    for b in T_ORDER:
        stats(b)
        tiny(b)
        if b != T_ORDER[-1]:
            ts_pass(b, "dve")   # teacher-phase DVE is idle
    # last teacher sample's TS runs on ACT (idle once its squares are
    # done ~51us, long before this sample's student chunks at ~81us),
    # keeping the student-phase DVE a pure KD pipeline.
    ts_pass(T_ORDER[-1], "act")
    for b in S_ORDER:
        loss(b)
    for b in S_ORDER:
        final(b)

    nc.sync.dma_start(out_ap[:, :], staging[:, :])


_CACHED = {}


def _get_nc():
    if "nc" in _CACHED:
        return _CACHED["nc"]
    nc = bacc.Bacc(
        "TRN2",
        target_bir_lowering=False,
        debug=False,
        enable_asserts=False,
        num_devices=N_CORES,
    )
    teacher = nc.dram_tensor("teacher", [BPC, P, FD], f32, kind="ExternalInput").ap()
    stu = nc.dram_tensor("stu", [BPC, P, FD], f32, kind="ExternalInput").ap()
    out = nc.dram_tensor("out", [1, 8 * BPC], f32, kind="ExternalOutput").ap()
    with tile.TileContext(nc) as tc:
        with ExitStack() as ctx:
            _build_kernel(ctx, tc, out, teacher, stu)
    nc.compile()
    _CACHED["nc"] = nc
    return nc


def _combine(parts):
    """parts: list of 8 arrays [1, 8*BPC] -> scalar loss."""
    losses = []
    for r in parts:
        r = np.asarray(r, dtype=np.float64).reshape(BPC, 8)
        losses.append(0.25 * r[:, 0])
    losses = np.concatenate(losses)
    return np.float32(LOSS_WEIGHT * losses.mean())


def run(inputs: dict, trace: bool = False):
    teacher = np.ascontiguousarray(np.asarray(inputs["teacher_feat"], dtype=np.float32))
    stu = np.ascontiguousarray(np.asarray(inputs["stu_feat"], dtype=np.float32))
    assert teacher.shape == (B, C, H, W) and stu.shape == (B, C, H, W)
    tch = teacher.reshape(N_CORES, BPC, P, FD)
    sch = stu.reshape(N_CORES, BPC, P, FD)
    in_maps = [
        {"teacher": np.ascontiguousarray(tch[i]), "stu": np.ascontiguousarray(sch[i])}
        for i in range(N_CORES)
    ]
    nc = _get_nc()
    res = run_bass_kernel_spmd(nc, in_maps, core_ids=list(range(N_CORES)), trace=trace)
    parts = [res.results[i]["out"] for i in range(N_CORES)]
    return _combine(parts), res


def kernel(**inputs) -> np.ndarray:
    out, _ = run(inputs, trace=False)
    return np.asarray(out, dtype=np.float32)


if __name__ == "__main__":
    rng = np.random.default_rng(0)
    ins = {
        "teacher_feat": rng.standard_normal((B, C, H, W), dtype=np.float32),
        "stu_feat": rng.standard_normal((B, C, H, W), dtype=np.float32),
    }
    print(kernel(**ins))


# revision 25
# speedup vs baseline: 1.1999x; 1.1999x over previous
"""KD feature-level smooth-L1 loss kernel for Trainium2 (8 NeuronCores).

Math (per batch sample b over (C,H,W) = 256*64*64 = N elements):
  t_norm = (t - mean) * rsqrt(var + eps)          # LayerNorm, no affine
  d   = |t_norm - s|
  kd  = where(d <= 2, d*d/4, d - 1)               # smooth-L1, beta=2
  out = mean_b( sum_chw(kd) )

Schedule: the kernel is a single ordered HBM stream on the SWDGE queue
(~33.6 MB/core read at ~420 GB/s, fp32->bf16 cast in the DMA datapath).
All four teacher tiles stream FIRST (sample 3 leading, so its LayerNorm
scalars are ready ~23 us in); per-sample stats run as chunks arrive
(sum(t) via bf16 ones-matmuls on PE, sum(t^2) via ACT Square with free
accumulation).  The normalize pass tn = t*rs + (-mean*rs) runs on the
DVE (tensor_scalar, 2x bf16 rate) for samples 3,0,1 during the teacher
phase while the DVE is otherwise idle, and on ACT (Identity with
per-partition scale/bias APs) for the last sample 2, filling ACT's
idle window after its squares.  Student chunks stream second (order
s3,s0,s1,s2) and the student phase is pure KD work on the DVE:
  y = tn - s ; c = clamp(y, -2, 2) ; 4*kd = c*(2y - c)   (+ accum)
tracking chunk arrivals with ~2 us of slack; the last sample's final
chunks are halved so only ~1 us of KD trails the last input byte.
Sharding: pure data parallel, 4 samples per core; host combines.
"""

from contextlib import ExitStack
from operator import add as _operator_add

import numpy as np

import concourse.mybir as mybir
import concourse.tile as tile
from concourse import bacc
from concourse.bass_utils import run_bass_kernel_spmd

B, C, H, W = 32, 256, 64, 64
N_CORES = 8
BPC = B // N_CORES            # samples per core
P = 128
N = C * H * W                 # 1048576 elements per sample
FD = N // P                   # 8192 free-dim per partition
HF = FD // 2                  # teacher DMA half width
SC = 2048                     # student chunk width (4 per sample)
MM = 512                      # matmul free-dim block (PSUM bank width)
EPS = 1e-5
BETA = 2.0
LOSS_WEIGHT = 1.0

f32 = mybir.dt.float32
bf16 = mybir.dt.bfloat16
AF = mybir.ActivationFunctionType
OP = mybir.AluOpType
AX = mybir.AxisListType

T_ORDER = [3, 0, 1, 2]        # teacher stream order (sample 3 first)
# student stream leads with the FIRST teacher sample, whose LayerNorm
# scalars have been ready for ~20 us by the time its chunks arrive.
S_ORDER = [3, 0, 1, 2]
# per-sample student chunk widths; last sample ends with two half chunks
S_CHUNKS = {3: [SC] * 4, 0: [SC] * 4, 1: [SC] * 4, 2: [SC, SC, SC, SC // 2, SC // 2]}


# ---------------- custom fused DVE op: accum += 4*kd ----------------------
def _register_kd_op():
    import concourse.dve_ops as dve_ops
    from concourse.dve_ops import DveOp
    from concourse.dve_spec import (
        C2,
        Latch,
        Spec,
        Src0,
        Src1,
        Zero,
        _has_src1,
        lower,
        maxx,
        minn,
    )
    from concourse.dve_table_gen import dve_ver_for
    from concourse.dve_uop import DveOpSpec

    name = "TENSOR_KD_SMOOTHL1"
    if name in dve_ops._SUB_OPCODE_FOR_NAME:
        return next(op for op in dve_ops.OPS if op.name == name)

    # in0 = tn (= t*rs - mean*rs, prescaled), in1 = s.
    # y = tn - s ; c = clamp(y, -2, 2) ; body = c*(2y - c) = 4*kd
    # 6 ALU ops + 1 accum stage <= 8-stage DVE pipeline.
    y = Src0 - Src1
    c = maxx(minn(y, C2), Latch(Zero - C2))
    body = c * (y + y - c)

    def _ref(in0, in1, c0, c1, c2):
        yv = in0.astype(np.float32) - in1.astype(np.float32)
        cv = np.clip(yv, -c2, c2)
        b = (cv * (2.0 * yv - cv)).astype(np.float32)
        return b, b.reshape(b.shape[0], -1).sum(axis=-1, keepdims=True)

    spec = Spec(body=body, accum=_operator_add, reference=_ref)
    ver = dve_ver_for("TRN2")
    row = max(dve_ops._SUB_OPCODE_FOR_NAME.values()) + 1
    assert row < 0x20
    probe = DveOpSpec(
        name=name, opcode=row, uops=lower(spec, ver=ver), rd1_en=_has_src1(spec)
    )
    op = DveOp(name, spec, subdim=False, uops_sha={ver: probe.sha(ver)})
    dve_ops.OPS.append(op)
    dve_ops.CUSTOM_DVE_SPECS[name] = spec
    dve_ops._SUB_OPCODE_FOR_NAME[name] = row
    return op


KD_OP = _register_kd_op()


def _build_kernel(ctx: ExitStack, tc: "tile.TileContext", out_ap, teacher, stu):
    nc = tc.nc

    const_pool = ctx.enter_context(tc.tile_pool(name="const", bufs=1))
    t_pool = ctx.enter_context(tc.tile_pool(name="t", bufs=BPC))
    tn_pool = ctx.enter_context(tc.tile_pool(name="tn", bufs=BPC))
    s_pool = ctx.enter_context(tc.tile_pool(name="s", bufs=8))
    dead_pool = ctx.enter_context(tc.tile_pool(name="dead", bufs=1))
    kdout_pool = ctx.enter_context(tc.tile_pool(name="kdout", bufs=1))
    sums_pool = ctx.enter_context(tc.tile_pool(name="sums", bufs=BPC))
    tiny_pool = ctx.enter_context(tc.tile_pool(name="tiny", bufs=BPC))
    bc_pool = ctx.enter_context(tc.tile_pool(name="bc", bufs=BPC))
    ps_t_pool = ctx.enter_context(tc.tile_pool(name="ps_t", bufs=2, space="PSUM"))
    ps_sq_pool = ctx.enter_context(tc.tile_pool(name="ps_sq", bufs=2, space="PSUM"))
    ps_kd_pool = ctx.enter_context(tc.tile_pool(name="ps_kd", bufs=2, space="PSUM"))
    ps_bc_pool = ctx.enter_context(tc.tile_pool(name="ps_bc", bufs=2, space="PSUM"))

    ones_bf = const_pool.tile([P, 1], bf16)
    nc.vector.memset(ones_bf[:], 1.0)
    ones_f32 = const_pool.tile([P, 1], f32)
    nc.vector.memset(ones_f32[:], 1.0)
    ones_row = const_pool.tile([1, P], f32)
    nc.vector.memset(ones_row[:], 1.0)
    staging = const_pool.tile([1, 8 * BPC], f32)
    nc.vector.memset(staging[:], 0.0)
    warm = const_pool.tile([1, 2], f32)
    # touch Sqrt first so its ACT table set (which also holds Square and
    # Identity) is loaded once at kernel start, off the critical path
    nc.scalar.activation(warm[0:1, 0:1], ones_f32[0:1, 0:1], AF.Sqrt)
    nc.scalar.activation(warm[0:1, 1:2], ones_f32[0:1, 0:1], AF.Identity)

    # ---------------- input DMAs up front, ONE ordered SWDGE stream ------
    # teacher first (sample stats must complete before any loss work),
    # sample 3 leading so its scalars are ready long before the student
    # phase; then student chunks in the same sample order.
    t_tiles = {}
    for b in T_ORDER:
        t_sb = t_pool.tile([P, FD], bf16, name="t")
        # last teacher tile streams in quarters so its stats lag the
        # stream by one quarter, not one half
        nq = 4 if b == T_ORDER[-1] else 2
        qw = FD // nq
        for q in range(nq):
            nc.gpsimd.dma_start(
                t_sb[:, q * qw : (q + 1) * qw], teacher[b, :, q * qw : (q + 1) * qw]
            )
        t_tiles[b] = t_sb
    s_bufs = {}
    for b in S_ORDER:
        off = 0
        for c, w in enumerate(S_CHUNKS[b]):
            s_sb = s_pool.tile([P, SC], bf16, name="s")
            nc.gpsimd.dma_start(s_sb[:, 0:w], stu[b, :, off : off + w])
            s_bufs[(b, c)] = s_sb
            off += w

    state = {}

    def stats(b):
        t_sb = t_tiles[b]
        ps_t = ps_t_pool.tile([1, MM], f32)
        nmm = FD // MM
        for k in range(nmm):
            nc.tensor.matmul(
                ps_t[:, :],
                ones_bf[:, :],
                t_sb[:, k * MM : (k + 1) * MM],
                start=(k == 0),
                stop=(k == nmm - 1),
            )
        # sums cols 0:4 = per-chunk sum(t^2), 4:4+nch = sum(4kd) chunks
        sums = sums_pool.tile([P, 12], f32)
        for ci in range(4):
            sl = slice(ci * SC, (ci + 1) * SC)
            dead = dead_pool.tile([P, SC], bf16)
            nc.scalar.activation(
                dead[:], t_sb[:, sl], AF.Square, accum_out=sums[:, ci : ci + 1]
            )
        state[b] = {"ps_t": ps_t, "sums": sums}

    def tiny(b):
        st_ = state[b]
        bb = tiny_pool.tile([1, 24], f32)
        ps_sq = ps_sq_pool.tile([1, 4], f32)
        nc.tensor.matmul(
            ps_sq[:, :], ones_f32[:, :], st_["sums"][:, 0:4], start=True, stop=True
        )
        st = bb[0:1, 3:4]
        nc.vector.reduce_sum(out=st, in_=st_["ps_t"][:, :], axis=AX.X)
        stt = bb[0:1, 4:5]
        nc.vector.reduce_sum(out=stt, in_=ps_sq[0:1, 0:4], axis=AX.X)
        mean = bb[0:1, 2:3]
        nc.vector.tensor_scalar(mean, st, 1.0 / N, None, op0=OP.mult)
        e2 = bb[0:1, 5:6]
        nc.vector.tensor_scalar(e2, stt, 1.0 / N, EPS, op0=OP.mult, op1=OP.add)
        msq = bb[0:1, 6:7]
        nc.vector.tensor_tensor(msq, mean, mean, op=OP.mult)
        ve = bb[0:1, 7:8]
        nc.vector.tensor_tensor(ve, e2, msq, op=OP.subtract)
        inv_ve = bb[0:1, 8:9]
        nc.vector.reciprocal(inv_ve, ve)
        rs = bb[0:1, 9:10]
        nc.scalar.activation(rs, inv_ve, AF.Sqrt)  # rs0 ~= 1/sqrt(ve) (table)
        # one Newton iteration: rs <- rs*(1.5 - 0.5*ve*rs^2)
        r2 = bb[0:1, 10:11]
        nc.vector.tensor_tensor(r2, rs, rs, op=OP.mult)
        pv = bb[0:1, 11:12]
        nc.vector.tensor_tensor(pv, r2, ve, op=OP.mult)
        hh = bb[0:1, 12:13]
        nc.vector.tensor_scalar(hh, pv, -0.5, 1.5, op0=OP.mult, op1=OP.add)
        rs_f = bb[0:1, 0:1]
        nc.vector.tensor_tensor(rs_f, rs, hh, op=OP.mult)
        # bb col0 = rs (final); col1 = -mean*rs
        mean_rs = bb[0:1, 13:14]
        nc.vector.tensor_tensor(mean_rs, mean, rs_f, op=OP.mult)
        nc.vector.tensor_scalar(bb[0:1, 1:2], mean_rs, -1.0, None, op0=OP.mult)
        # broadcast rs / -mean*rs to all partitions via a PE ones-row
        # matmul, then land them in SBUF (ACT scale/bias must be SBUF APs)
        ps_bc = ps_bc_pool.tile([P, 2], f32)
        nc.tensor.matmul(
            ps_bc[:, :], ones_row[:, :], bb[0:1, 0:2], start=True, stop=True
        )
        bc_sb = bc_pool.tile([P, 2], f32)
        nc.vector.tensor_copy(bc_sb[:], ps_bc[:])
        st_["rs_vec"] = bc_sb[:, 0:1]
        st_["nmrs_vec"] = bc_sb[:, 1:2]

    def ts_pass(b, engine):
        # tn = t*rs + (-mean*rs), per student-chunk slice
        st_ = state[b]
        t_sb = t_tiles[b]
        tn = tn_pool.tile([P, FD], bf16, name="tn")
        for c in range(4):
            sl = slice(c * SC, (c + 1) * SC)
            if engine == "act":
                nc.scalar.activation(
                    tn[:, sl],
                    t_sb[:, sl],
                    AF.Identity,
                    bias=st_["nmrs_vec"],
                    scale=st_["rs_vec"],
                )
            else:
                nc.vector.tensor_scalar(
                    tn[:, sl],
                    t_sb[:, sl],
                    st_["rs_vec"],
                    st_["nmrs_vec"],
                    op0=OP.mult,
                    op1=OP.add,
                )
        st_["tn"] = tn

    def loss(b):
        st_ = state[b]
        sums = st_["sums"]
        off = 0
        for c, w in enumerate(S_CHUNKS[b]):
            kdo = kdout_pool.tile([P, SC], bf16, name="kdo")
            nc.vector._custom_dve(
                KD_OP,
                out=kdo[:, 0:w],
                in0=st_["tn"][:, off : off + w],
                in1=s_bufs[(b, c)][:, 0:w],
                imm2=BETA,
                accum_out=sums[:, 4 + c : 5 + c],
            )
            off += w

    def final(b):
        st_ = state[b]
        nch = len(S_CHUNKS[b])
        ps_kd = ps_kd_pool.tile([1, 8], f32)
        nc.tensor.matmul(
            ps_kd[:, 0:nch],
            ones_f32[:, :],
            st_["sums"][:, 4 : 4 + nch],
            start=True,
            stop=True,
        )
        nc.vector.reduce_sum(
            out=staging[0:1, 8 * b : 8 * b + 1], in_=ps_kd[0:1, 0:nch], axis=AX.X
        )

    for b in T_ORDER:
        stats(b)
        tiny(b)
        if b != T_ORDER[-1]:
            ts_pass(b, "dve")   # teacher-phase DVE is idle
    # last teacher sample's TS runs on ACT (idle once its squares are
    # done ~51us, long before this sample's student chunks at ~81us),
    # keeping the student-phase DVE a pure KD pipeline.
    ts_pass(T_ORDER[-1], "act")
    for b in S_ORDER:
        loss(b)
    for b in S_ORDER:
        final(b)

    nc.sync.dma_start(out_ap[:, :], staging[:, :])


_CACHED = {}


def _get_nc():
    if "nc" in _CACHED:
        return _CACHED["nc"]
    nc = bacc.Bacc(
        "TRN2",
        target_bir_lowering=False,
        debug=False,
        enable_asserts=False,
        num_devices=N_CORES,
    )
    teacher = nc.dram_tensor("teacher", [BPC, P, FD], f32, kind="ExternalInput").ap()
    stu = nc.dram_tensor("stu", [BPC, P, FD], f32, kind="ExternalInput").ap()
    out = nc.dram_tensor("out", [1, 8 * BPC], f32, kind="ExternalOutput").ap()
    with tile.TileContext(nc) as tc:
        with ExitStack() as ctx:
            _build_kernel(ctx, tc, out, teacher, stu)
    nc.compile()
    _CACHED["nc"] = nc
    return nc


def _combine(parts):
    """parts: list of 8 arrays [1, 8*BPC] -> scalar loss."""
    losses = []
    for r in parts:
        r = np.asarray(r, dtype=np.float64).reshape(BPC, 8)
        losses.append(0.25 * r[:, 0])
    losses = np.concatenate(losses)
    return np.float32(LOSS_WEIGHT * losses.mean())


def run(inputs: dict, trace: bool = False):
    teacher = np.ascontiguousarray(np.asarray(inputs["teacher_feat"], dtype=np.float32))
    stu = np.ascontiguousarray(np.asarray(inputs["stu_feat"], dtype=np.float32))
    assert teacher.shape == (B, C, H, W) and stu.shape == (B, C, H, W)
    tch = teacher.reshape(N_CORES, BPC, P, FD)
    sch = stu.reshape(N_CORES, BPC, P, FD)
    in_maps = [
        {"teacher": np.ascontiguousarray(tch[i]), "stu": np.ascontiguousarray(sch[i])}
        for i in range(N_CORES)
    ]
    nc = _get_nc()
    res = run_bass_kernel_spmd(nc, in_maps, core_ids=list(range(N_CORES)), trace=trace)
    parts = [res.results[i]["out"] for i in range(N_CORES)]
    return _combine(parts), res


def kernel(**inputs) -> np.ndarray:
    out, _ = run(inputs, trace=False)
    return np.asarray(out, dtype=np.float32)


if __name__ == "__main__":
    rng = np.random.default_rng(0)
    ins = {
        "teacher_feat": rng.standard_normal((B, C, H, W), dtype=np.float32),
        "stu_feat": rng.standard_normal((B, C, H, W), dtype=np.float32),
    }
    print(kernel(**ins))
